# revision 1
# baseline (speedup 1.0000x reference)
"""Trainium2 Bass kernel for DCTEncoderLayer.

Computes, for rgb_images_batch [32, 3, 512, 512] f32:
  ycbcr' = 2*rgb_to_ycbcr(rgb) - 1                 (per-pixel 3x3 channel mix, affine)
  32x32 block DCT per channel, coefficients scaled by (2/32)*c_u*c_v,
  output [32, 3*1024, 16, 16] with the frequency axis sorted by |(v,u)|.

Strategy (pure data parallel over batch, 4 images per NeuronCore):
  The 2D DCT is separable: coeff = Cs @ block @ Cs.T with Cs[v,y] =
  cos((2y+1)v*pi/64) * c_v / 4.  The YCbCr channel mix is linear and is
  folded into the stage-1 weights (contraction runs over (channel, y));
  feeding the device rgb-0.5 makes the affine offset exact (the shifted
  input has zero offset in every output channel).
  Per (image, block-row) iteration on device:
    stage1: t1[(c,v), x]       = W1m.T @ img[(c',y), x]     (matmul, N=512)
    stream_transpose (DVE):    tbt[(c,x'), (gx,v)]          (32x32 blockwise,
                               exactly what the block-diagonal stage-2 needs)
    round to f32r:             tbr = tbt                    (DVE/ACT copy)
    stage2: out[(c,u),(gx,v)]  = W2bd.T @ tbr               (matmul, N=512)
  Matmuls run in float16 (10-bit mantissa like TF32, ~4e-4 rel err,
  4x faster than fp32 on the PE, half the input DMA bytes).  The device writes raw [64, 96, 512] tiles; the host
  reassembles/permutes axes and applies the frequency sort.
"""

import os
import sys

try:
    import concourse.bass  # noqa: F401
except ImportError:  # bare interpreter without the axon site paths
    sys.path.insert(0, "/opt/trn_rl_repo")

import numpy as np

import concourse.bacc as bacc
import concourse.bass as bass
import concourse.mybir as mybir
import concourse.tile as tile
from concourse.bass_utils import run_bass_kernel_spmd

F32 = mybir.dt.float32
F32R = mybir.dt.float32r
F16 = mybir.dt.float16

BS = 32            # DCT block size
N_CORES = 8
B_PER_CORE = 4     # batch images per core
NH = 16            # blocks per row/column (512/32)
ITERS = B_PER_CORE * NH  # 64 per core

_STATE = {}
LAST_RESULT = None  # BassKernelResults of the most recent run (for profiling)


def _dct_mat():
    """Cs[v, y] = cos((2y+1) v pi / 64) * c_v / 4  (f64)."""
    y = np.arange(BS)
    v = np.arange(BS)[:, None]
    c = np.cos((2 * y + 1) * v * np.pi / (2 * BS))
    c[0, :] *= 1.0 / np.sqrt(2.0)
    return c / 4.0


def _sort_idx():
    # must replicate the reference's argsort (default kind) exactly,
    # including its tie order for equal |(v,u)|
    mag = np.zeros((BS, BS), dtype=np.float64)
    for v in range(BS):
        for u in range(BS):
            mag[v, u] = np.linalg.norm(np.array([v, u], dtype=np.int64))
    return np.argsort(mag.reshape(-1))


def _constants():
    cs = _dct_mat()
    # rows (y', cb', cr') of the linear part of 2*rgb_to_ycbcr(rgb)-1, in (r,g,b)
    a2 = np.array(
        [
            [2 * 0.299, 2 * 0.587, 2 * 0.114],
            [2 * 0.564 * -0.299, 2 * 0.564 * -0.587, 2 * 0.564 * (1 - 0.114)],
            [2 * 0.713 * (1 - 0.299), 2 * 0.713 * -0.587, 2 * 0.713 * -0.114],
        ],
        np.float64,
    )
    w1 = np.zeros((96, 96))  # [(c', y), (c, v)]
    for cp in range(3):
        for c in range(3):
            w1[cp * 32 : (cp + 1) * 32, c * 32 : (c + 1) * 32] = a2[c, cp] * cs.T
    w2 = np.zeros((96, 96))  # [(c, x'), (c, u)] block diagonal over c
    for c in range(3):
        w2[c * 32 : (c + 1) * 32, c * 32 : (c + 1) * 32] = cs.T
    return w1.astype(np.float16), w2.astype(np.float16)


def _build_program():
    nc = bacc.Bacc(trn_type="TRN2")
    x = nc.dram_tensor("x", [B_PER_CORE, NH, 96, 512], F16, kind="ExternalInput")
    w1 = nc.dram_tensor("w1", [96, 96], F16, kind="ExternalInput")
    w2 = nc.dram_tensor("w2", [96, 96], F16, kind="ExternalInput")
    # 32 mega-iterations of 2 block-rows each
    out = nc.dram_tensor("out", [ITERS // 2, 96, 1024], F32, kind="ExternalOutput")

    with tile.TileContext(nc) as tc:
        with (
            tc.tile_pool(name="const", bufs=1) as constp,
            tc.tile_pool(name="sb", bufs=5) as sb,
            tc.tile_pool(name="psA", bufs=2, space="PSUM") as psA,
            tc.tile_pool(name="psB", bufs=2, space="PSUM") as psB,
        ):
            w1s = constp.tile([96, 96], F16)
            w2s = constp.tile([96, 96], F16)
            nc.sync.dma_start(w1s[:], w1[:])
            nc.sync.dma_start(w2s[:], w2[:])

            for it in range(ITERS // 2):
                b, brr = it // (NH // 2), it % (NH // 2)
                img = sb.tile([96, 1024], F16, tag="img")
                nc.sync.dma_start(
                    img[:].rearrange("p (r x) -> p r x", r=2),
                    x[b, brr * 2 : brr * 2 + 2, :, :].rearrange("r p x -> p r x"),
                )
                # stage 1: t1[(c,v), (r2, x)] = W1m.T @ img
                t1p = psA.tile([96, 1024], F32, tag="t1p")
                for h in range(2):
                    nc.tensor.matmul(
                        t1p[:, h * 512 : (h + 1) * 512],
                        w1s[:],
                        img[:, h * 512 : (h + 1) * 512],
                        start=True,
                        stop=True,
                    )
                # 32x32 blockwise transpose: tbt[(c,x'), (r2, gx, v)]
                tbt = sb.tile([96, 1024], F32, tag="tbt")
                nc.vector.transpose(tbt[:], t1p[:])
                # round to fp16 for stage 2 (1/2 GpSimd, 1/4 DVE, 1/4 ACT)
                tbr = sb.tile([96, 1024], F16, tag="tbr")
                if it % 2 == 0:
                    nc.gpsimd.tensor_copy(tbr[:], tbt[:])
                elif it % 4 == 1:
                    nc.vector.tensor_copy(tbr[:], tbt[:])
                else:
                    nc.scalar.copy(tbr[:], tbt[:])
                # stage 2: out2[(c,u), (r2, gx, v)] = W2bd.T @ tbt
                o2p = psB.tile([96, 1024], F32, tag="o2p")
                for h in range(2):
                    nc.tensor.matmul(
                        o2p[:, h * 512 : (h + 1) * 512],
                        w2s[:],
                        tbr[:, h * 512 : (h + 1) * 512],
                        start=True,
                        stop=True,
                    )
                osb = sb.tile([96, 1024], F32, tag="osb")
                if it % 8 == 0:
                    nc.vector.tensor_copy(osb[:], o2p[:])
                else:
                    nc.scalar.copy(osb[:], o2p[:])
                nc.sync.dma_start(out[it], osb[:])

    nc.finalize()
    return nc


def _get_program():
    if "nc" not in _STATE:
        _STATE["nc"] = _build_program()
        _STATE["consts"] = _constants()
        _STATE["sort_idx"] = _sort_idx()
    return _STATE["nc"]


def kernel(**inputs):
    global LAST_RESULT
    rgb = np.asarray(inputs["rgb_images_batch"], np.float32)
    assert rgb.shape == (N_CORES * B_PER_CORE, 3, 512, 512)
    # centering makes the YCbCr affine offset vanish (row sums of the cb/cr
    # mix are 0 and the y row sums to 2 -> offset 2*0.5-1=0 for every channel)
    xs = rgb.reshape(N_CORES * B_PER_CORE, 3, NH, 32, 512).transpose(0, 2, 1, 3, 4)
    xs = (np.ascontiguousarray(xs).reshape(N_CORES * B_PER_CORE, NH, 96, 512)
          - np.float32(0.5)).astype(np.float16)
    nc = _get_program()
    w1, w2 = _STATE["consts"]
    sort_idx = _STATE["sort_idx"]

    in_maps = [
        {"x": xs[c * B_PER_CORE : (c + 1) * B_PER_CORE], "w1": w1, "w2": w2}
        for c in range(N_CORES)
    ]
    trace = os.environ.get("KERNEL_TRACE", "0") == "1"
    res = run_bass_kernel_spmd(
        nc, in_maps, core_ids=list(range(N_CORES)), trace=trace
    )
    LAST_RESULT = res

    outs = []
    for c in range(N_CORES):
        dev = res.results[c]["out"]  # [32, 96, 1024]
        a = dev.reshape(B_PER_CORE, NH // 2, 3, 32, 2, NH, 32)  # b,brr,c,u,r2,gx,v
        a = a.transpose(0, 2, 6, 3, 1, 4, 5)  # b, c, v, u, brr, r2, gx
        a = np.ascontiguousarray(a).reshape(B_PER_CORE, 3, 1024, NH, NH)
        a = a[:, :, sort_idx, :, :]
        outs.append(a.reshape(B_PER_CORE, 3 * 1024, NH, NH))
    return np.concatenate(outs, axis=0)



# revision 7
# speedup vs baseline: 1.4105x; 1.4105x over previous
"""Trainium2 Bass kernel for DCTEncoderLayer — v3 "stationary swap".

Same math as v2 (separable 32x32 DCT, YCbCr mix folded into stage-1
weights), but stage 1 runs with the IMAGE as the PE stationary operand:

    t1T[x, (c,v)] = img_chunk[(c',y), x].T @ W1[(c',y), (c,v)]

which lands the stage-1 result already transposed (x on partitions) —
no DVE stream-transpose is needed anywhere.  Stage 2 is then a single
128-partition block-diagonal DCT along x:

    out[(gxl,u), (k,c,v)] = W2bd[(gxl,x'), (gxl,u)].T @ t1s[(gxl,x'), (k,c,v)]

Per [96, 1024] iteration (2 block-rows):
  - 8 stage-1 matmuls: stationary img chunk [96,128] fp16, moving W1 [96,96]
  - cast1 (ACT/DVE): t1T PSUM f32 [128, 8x(96 of 128)] -> SBUF fp16 [128,768]
  - 3 stage-2 matmuls: stationary W2bd [128,128] fp16, moving [128,256] fp16
  - cast2 (ACT/DVE): o2p PSUM f32 -> SBUF fp16
  - DMA out fp16 (halved output bytes); host upcasts + permutes + freq-sorts.
"""

import os
import sys

try:
    import concourse.bass  # noqa: F401
except ImportError:
    sys.path.insert(0, "/opt/trn_rl_repo")

import numpy as np

import concourse.bacc as bacc
import concourse.bass as bass
import concourse.mybir as mybir
import concourse.tile as tile
from concourse.bass_utils import run_bass_kernel_spmd

F32 = mybir.dt.float32
F16 = mybir.dt.float16

BS = 32
N_CORES = 8
B_PER_CORE = 4
NH = 16
ITERS = B_PER_CORE * NH  # 64 block-rows per core; fused 2/iter -> 32 iters

_STATE = {}
LAST_RESULT = None


def _dct_mat():
    y = np.arange(BS)
    v = np.arange(BS)[:, None]
    c = np.cos((2 * y + 1) * v * np.pi / (2 * BS))
    c[0, :] *= 1.0 / np.sqrt(2.0)
    return c / 4.0


def _sort_idx():
    mag = np.zeros((BS, BS), dtype=np.float64)
    for v in range(BS):
        for u in range(BS):
            mag[v, u] = np.linalg.norm(np.array([v, u], dtype=np.int64))
    return np.argsort(mag.reshape(-1))


def _constants():
    cs = _dct_mat()
    a2 = np.array(
        [
            [2 * 0.299, 2 * 0.587, 2 * 0.114],
            [2 * 0.564 * -0.299, 2 * 0.564 * -0.587, 2 * 0.564 * (1 - 0.114)],
            [2 * 0.713 * (1 - 0.299), 2 * 0.713 * -0.587, 2 * 0.713 * -0.114],
        ],
        np.float64,
    )
    w1 = np.zeros((96, 96))  # [(c', y), (c, v)]
    for cp in range(3):
        for c in range(3):
            w1[cp * 32 : (cp + 1) * 32, c * 32 : (c + 1) * 32] = a2[c, cp] * cs.T
    w2 = np.zeros((128, 128))  # [(gxl, x'), (gxl, u)] block diagonal over gxl
    for g in range(4):
        w2[g * 32 : (g + 1) * 32, g * 32 : (g + 1) * 32] = cs.T
    return w1.astype(np.float16), w2.astype(np.float16)


def _build_program():
    nc = bacc.Bacc(trn_type="TRN2")
    x = nc.dram_tensor("x", [B_PER_CORE, NH, 96, 512], F16, kind="ExternalInput")
    w1 = nc.dram_tensor("w1", [96, 96], F16, kind="ExternalInput")
    w2 = nc.dram_tensor("w2", [128, 128], F16, kind="ExternalInput")
    out = nc.dram_tensor("out", [ITERS // 2, 128, 768], F16, kind="ExternalOutput")

    with tile.TileContext(nc) as tc:
        with (
            tc.tile_pool(name="const", bufs=1) as constp,
            tc.tile_pool(name="sb", bufs=5) as sb,
            tc.tile_pool(name="psA", bufs=2, space="PSUM") as psA,
            tc.tile_pool(name="psB", bufs=2, space="PSUM") as psB,
        ):
            w1s = constp.tile([96, 96], F16)
            w2s = constp.tile([128, 128], F16)
            nc.sync.dma_start(w1s[:], w1[:])
            nc.sync.dma_start(w2s[:], w2[:])

            for it in range(ITERS // 2):
                b, brr = it // (NH // 2), it % (NH // 2)
                img = sb.tile([96, 1024], F16, tag="img")
                nc.sync.dma_start(
                    img[:].rearrange("p (r x) -> p r x", r=2),
                    x[b, brr * 2 : brr * 2 + 2, :, :].rearrange("r p x -> p r x"),
                )
                # stage 1 (stationary swap): t1T[x128, (c,v)] = chunk.T @ W1
                # chunk slots padded to 128 f32 so no slot crosses a PSUM
                # bank.  start=True zeroes the whole 2KB zero-region (bank),
                # so only the first matmul of each bank may set it; the rest
                # accumulate into lazily-zeroed bytes of the open group.
                t1T = psA.tile([128, 1024], F32, tag="t1T")
                for k in range(8):
                    nc.tensor.matmul(
                        t1T[:, k * 128 : k * 128 + 96],
                        img[:, k * 128 : (k + 1) * 128],
                        w1s[:],
                        start=(k % 4 == 0),
                        stop=(k % 4 == 3),
                        skip_group_check=True,
                    )
                # cast1: pack the 8 chunk slots -> contiguous fp16 [128, 768]
                t1s = sb.tile([128, 768], F16, tag="t1s")
                src = t1T[:].rearrange("p (k s) -> p k s", k=8)[:, :, 0:96]
                dst = t1s[:].rearrange("p (k s) -> p k s", k=8)
                if it % 2 == 0:
                    nc.scalar.copy(dst, src)
                else:
                    nc.vector.tensor_copy(dst, src)
                # stage 2: block-diag DCT along x', 128 partitions.
                # allocate a full 2-bank tile so double-buffered slots stay
                # bank-aligned; cols 0-767 used.  h=0/h=1 share bank 0, so
                # h=1 must not re-zero it (start only on bank firsts).
                o2p = psB.tile([128, 1024], F32, tag="o2p")
                for h in range(3):
                    nc.tensor.matmul(
                        o2p[:, h * 256 : (h + 1) * 256],
                        w2s[:],
                        t1s[:, h * 256 : (h + 1) * 256],
                        start=(h != 1),
                        stop=(h != 0),
                        skip_group_check=True,
                    )
                # cast2 -> fp16 for output DMA
                osb = sb.tile([128, 768], F16, tag="osb")
                if it % 2 == 0:
                    nc.vector.tensor_copy(osb[:], o2p[:, 0:768])
                else:
                    nc.scalar.copy(osb[:], o2p[:, 0:768])
                nc.sync.dma_start(out[it], osb[:])

    nc.finalize()
    return nc


def _get_program():
    if "nc" not in _STATE:
        _STATE["nc"] = _build_program()
        _STATE["consts"] = _constants()
        _STATE["sort_idx"] = _sort_idx()
    return _STATE["nc"]


def kernel(**inputs):
    global LAST_RESULT
    rgb = np.asarray(inputs["rgb_images_batch"], np.float32)
    assert rgb.shape == (N_CORES * B_PER_CORE, 3, 512, 512)
    xs = rgb.reshape(N_CORES * B_PER_CORE, 3, NH, 32, 512).transpose(0, 2, 1, 3, 4)
    xs = (np.ascontiguousarray(xs).reshape(N_CORES * B_PER_CORE, NH, 96, 512)
          - np.float32(0.5)).astype(np.float16)
    nc = _get_program()
    w1, w2 = _STATE["consts"]
    sort_idx = _STATE["sort_idx"]

    in_maps = [
        {"x": xs[c * B_PER_CORE : (c + 1) * B_PER_CORE], "w1": w1, "w2": w2}
        for c in range(N_CORES)
    ]
    trace = os.environ.get("KERNEL_TRACE", "0") == "1"
    res = run_bass_kernel_spmd(
        nc, in_maps, core_ids=list(range(N_CORES)), trace=trace
    )
    LAST_RESULT = res

    outs = []
    for c in range(N_CORES):
        dev = res.results[c]["out"].astype(np.float32)  # [32, 128, 768]
        # [it=(b,brr), p=(gxl,u), col=(k=(r2,kk), c, v)]
        a = dev.reshape(B_PER_CORE, 8, 4, 32, 2, 4, 3, 32)  # b,brr,gxl,u,r2,kk,c,v
        a = a.transpose(0, 6, 7, 3, 1, 4, 5, 2)  # b,c,v,u,brr,r2,kk,gxl
        a = np.ascontiguousarray(a).reshape(B_PER_CORE, 3, 1024, NH, NH)
        a = a[:, :, sort_idx, :, :]
        outs.append(a.reshape(B_PER_CORE, 3 * 1024, NH, NH))
    return np.concatenate(outs, axis=0)


# revision 8
# speedup vs baseline: 1.5839x; 1.1230x over previous
"""Trainium2 Bass kernel for DCTEncoderLayer — v3.1 "stationary swap".

Separable 32x32 DCT with the YCbCr mix folded into stage-1 weights.
Stage 1 runs with the IMAGE as the PE stationary operand:

    t1T[x, (c,v)] = img_chunk[(c',y), x].T @ W1[(c',y), (c,v)]

which lands the stage-1 result already transposed (x on partitions) —
no DVE stream-transpose is needed anywhere.  Stage 2 is a single
128-partition block-diagonal DCT along x':

    out[(gxl,u), (k,c,v)] = W2bd[(gxl,x'), (gxl,u)].T @ t1s[(gxl,x'), (k,c,v)]

v3.1: one block-row [96,512] per iteration (64 iters) so each stage's
PSUM tile is a single bank and both PSUM pools run 4 deep — the
per-iteration dependency chain (DMA->4mm->cast->mm->cast->DMA, ~2.5us)
pipelines 4-wide instead of 2.  Output DMAs issue from the otherwise
idle GpSimd queue so they don't serialize behind input DMAs on Sync.

Per [96, 512] iteration:
  - 4 stage-1 matmuls: stationary img chunk [96,128] fp16, moving W1 [96,96]
  - cast1 (ACT/DVE alt): t1T PSUM f32 [128, 4x(96 of 128)] -> SBUF fp16 [128,384]
  - 1 stage-2 matmul: stationary W2bd [128,128] fp16, moving [128,384] fp16
  - cast2 (DVE/ACT alt): o2p PSUM f32 -> SBUF fp16 [128,384]
  - DMA out fp16 (halved output bytes); host upcasts + permutes + freq-sorts.
"""

import os
import sys

try:
    import concourse.bass  # noqa: F401
except ImportError:
    sys.path.insert(0, "/opt/trn_rl_repo")

import numpy as np

import concourse.bacc as bacc
import concourse.bass as bass
import concourse.mybir as mybir
import concourse.tile as tile
from concourse.bass_utils import run_bass_kernel_spmd

F32 = mybir.dt.float32
F16 = mybir.dt.float16

BS = 32
N_CORES = 8
B_PER_CORE = 4
NH = 16
ITERS = B_PER_CORE * NH  # 64 block-rows per core, one per iteration

_STATE = {}
LAST_RESULT = None


def _dct_mat():
    y = np.arange(BS)
    v = np.arange(BS)[:, None]
    c = np.cos((2 * y + 1) * v * np.pi / (2 * BS))
    c[0, :] *= 1.0 / np.sqrt(2.0)
    return c / 4.0


def _sort_idx():
    mag = np.zeros((BS, BS), dtype=np.float64)
    for v in range(BS):
        for u in range(BS):
            mag[v, u] = np.linalg.norm(np.array([v, u], dtype=np.int64))
    return np.argsort(mag.reshape(-1))


def _constants():
    cs = _dct_mat()
    a2 = np.array(
        [
            [2 * 0.299, 2 * 0.587, 2 * 0.114],
            [2 * 0.564 * -0.299, 2 * 0.564 * -0.587, 2 * 0.564 * (1 - 0.114)],
            [2 * 0.713 * (1 - 0.299), 2 * 0.713 * -0.587, 2 * 0.713 * -0.114],
        ],
        np.float64,
    )
    w1 = np.zeros((96, 96))  # [(c', y), (c, v)]
    for cp in range(3):
        for c in range(3):
            w1[cp * 32 : (cp + 1) * 32, c * 32 : (c + 1) * 32] = a2[c, cp] * cs.T
    w2 = np.zeros((128, 128))  # [(gxl, x'), (gxl, u)] block diagonal over gxl
    for g in range(4):
        w2[g * 32 : (g + 1) * 32, g * 32 : (g + 1) * 32] = cs.T
    return w1.astype(np.float16), w2.astype(np.float16)


def _build_program():
    nc = bacc.Bacc(trn_type="TRN2")
    x = nc.dram_tensor("x", [B_PER_CORE, NH, 96, 512], F16, kind="ExternalInput")
    w1 = nc.dram_tensor("w1", [96, 96], F16, kind="ExternalInput")
    w2 = nc.dram_tensor("w2", [128, 128], F16, kind="ExternalInput")
    out = nc.dram_tensor("out", [ITERS, 128, 384], F16, kind="ExternalOutput")

    with tile.TileContext(nc) as tc:
        with (
            tc.tile_pool(name="const", bufs=1) as constp,
            tc.tile_pool(name="sb", bufs=8) as sb,
            tc.tile_pool(name="psA", bufs=4, space="PSUM") as psA,
            tc.tile_pool(name="psB", bufs=4, space="PSUM") as psB,
        ):
            w1s = constp.tile([96, 96], F16)
            w2s = constp.tile([128, 128], F16)
            nc.sync.dma_start(w1s[:], w1[:])
            nc.sync.dma_start(w2s[:], w2[:])

            for it in range(ITERS):
                b, br = it // NH, it % NH
                img = sb.tile([96, 512], F16, tag="img")
                nc.sync.dma_start(img[:], x[b, br])
                # stage 1 (stationary swap): t1T[x128, (c,v)] = chunk.T @ W1
                # 4 chunk slots of 128 f32 in a single PSUM bank
                t1T = psA.tile([128, 512], F32, tag="t1T")
                for k in range(4):
                    nc.tensor.matmul(
                        t1T[:, k * 128 : k * 128 + 96],
                        img[:, k * 128 : (k + 1) * 128],
                        w1s[:],
                        start=True,
                        stop=True,
                    )
                # cast1: pack the 4 chunk slots -> contiguous fp16 [128, 384]
                t1s = sb.tile([128, 384], F16, tag="t1s")
                src = t1T[:].rearrange("p (k s) -> p k s", k=4)[:, :, 0:96]
                dst = t1s[:].rearrange("p (k s) -> p k s", k=4)
                if it % 2 == 0:
                    nc.scalar.copy(dst, src)
                else:
                    nc.vector.tensor_copy(dst, src)
                # stage 2: block-diag DCT along x', 128 partitions, one matmul
                o2p = psB.tile([128, 512], F32, tag="o2p")
                nc.tensor.matmul(
                    o2p[:, 0:384],
                    w2s[:],
                    t1s[:],
                    start=True,
                    stop=True,
                )
                # cast2 -> fp16 for the output DMA
                osb = sb.tile([128, 384], F16, tag="osb")
                if it % 2 == 0:
                    nc.vector.tensor_copy(osb[:], o2p[:, 0:384])
                else:
                    nc.scalar.copy(osb[:], o2p[:, 0:384])
                # issue output DMAs from the idle GpSimd queue so they don't
                # serialize behind the input DMAs on Sync
                nc.gpsimd.dma_start(out[it], osb[:])

    nc.finalize()
    return nc


def _get_program():
    if "nc" not in _STATE:
        _STATE["nc"] = _build_program()
        _STATE["consts"] = _constants()
        _STATE["sort_idx"] = _sort_idx()
    return _STATE["nc"]


def kernel(**inputs):
    global LAST_RESULT
    rgb = np.asarray(inputs["rgb_images_batch"], np.float32)
    assert rgb.shape == (N_CORES * B_PER_CORE, 3, 512, 512)
    xs = rgb.reshape(N_CORES * B_PER_CORE, 3, NH, 32, 512).transpose(0, 2, 1, 3, 4)
    xs = (np.ascontiguousarray(xs).reshape(N_CORES * B_PER_CORE, NH, 96, 512)
          - np.float32(0.5)).astype(np.float16)
    nc = _get_program()
    w1, w2 = _STATE["consts"]
    sort_idx = _STATE["sort_idx"]

    in_maps = [
        {"x": xs[c * B_PER_CORE : (c + 1) * B_PER_CORE], "w1": w1, "w2": w2}
        for c in range(N_CORES)
    ]
    trace = os.environ.get("KERNEL_TRACE", "0") == "1"
    res = run_bass_kernel_spmd(
        nc, in_maps, core_ids=list(range(N_CORES)), trace=trace
    )
    LAST_RESULT = res

    outs = []
    for c in range(N_CORES):
        dev = res.results[c]["out"].astype(np.float32)  # [64, 128, 384]
        # [it=(b,br), p=(gxl,u), col=(kk, c, v)]
        a = dev.reshape(B_PER_CORE, NH, 4, 32, 4, 3, 32)  # b,br,gxl,u,kk,c,v
        a = a.transpose(0, 5, 6, 3, 1, 4, 2)  # b,c,v,u,br,kk,gxl
        a = np.ascontiguousarray(a).reshape(B_PER_CORE, 3, 1024, NH, NH)
        a = a[:, :, sort_idx, :, :]
        outs.append(a.reshape(B_PER_CORE, 3 * 1024, NH, NH))
    return np.concatenate(outs, axis=0)


# revision 9
# speedup vs baseline: 1.8447x; 1.1646x over previous
"""Trainium2 Bass kernel for DCTEncoderLayer — v3.2 "stationary swap, batched DMA".

Separable 32x32 DCT with the YCbCr mix folded into stage-1 weights.
Stage 1 runs with the IMAGE as the PE stationary operand:

    t1T[x, (c,v)] = img_chunk[(c',y), x].T @ W1[(c',y), (c,v)]

which lands the stage-1 result already transposed (x on partitions) —
no DVE stream-transpose is needed anywhere.  Stage 2 is a single
128-partition block-diagonal DCT along x':

    out[(gxl,u), (kk,c,v)] = W2bd[(gxl,x'), (gxl,u)].T @ t1s[(gxl,x'), (kk,c,v)]

v3.2 vs v3.1: DMAs carry ~350ns fixed cost each, so input DMAs batch 4
block-rows (host lays x out so each partition reads 4KB contiguously)
and output DMAs batch 4 iterations' results; stage-2 PSUM pairs two
iterations per 2-bank tile so each cast2 covers 768 columns.  Output
DMAs issue from the otherwise idle GpSimd DGE queue, input from Sync.

Per block-row iteration (64 per core, grouped by 4):
  - 4 stage-1 matmuls: stationary img chunk [96,128] fp16, moving W1 [96,96]
  - cast1 (ACT/DVE alt): t1T PSUM f32 [128, 4x(96 of 128)] -> SBUF fp16 [128,384]
  - 1 stage-2 matmul: stationary W2bd [128,128] fp16, moving [128,384] fp16
  - per 2 iters: cast2 (ACT/DVE alt) o2p PSUM f32 -> fp16 into the group's
    staging tile; per 4 iters: one output DMA [128, 1536] fp16.
Host upcasts, permutes axes and applies the frequency sort.
"""

import os
import sys

try:
    import concourse.bass  # noqa: F401
except ImportError:
    sys.path.insert(0, "/opt/trn_rl_repo")

import numpy as np

import concourse.bacc as bacc
import concourse.bass as bass
import concourse.mybir as mybir
import concourse.tile as tile
from concourse.bass_utils import run_bass_kernel_spmd

F32 = mybir.dt.float32
F16 = mybir.dt.float16

BS = 32
N_CORES = 8
B_PER_CORE = 4
NH = 16
ITERS = B_PER_CORE * NH  # 64 block-rows per core
GROUPS = ITERS // 4      # 16 groups of 4 block-rows

_STATE = {}
LAST_RESULT = None


def _dct_mat():
    y = np.arange(BS)
    v = np.arange(BS)[:, None]
    c = np.cos((2 * y + 1) * v * np.pi / (2 * BS))
    c[0, :] *= 1.0 / np.sqrt(2.0)
    return c / 4.0


def _sort_idx():
    mag = np.zeros((BS, BS), dtype=np.float64)
    for v in range(BS):
        for u in range(BS):
            mag[v, u] = np.linalg.norm(np.array([v, u], dtype=np.int64))
    return np.argsort(mag.reshape(-1))


def _constants():
    cs = _dct_mat()
    a2 = np.array(
        [
            [2 * 0.299, 2 * 0.587, 2 * 0.114],
            [2 * 0.564 * -0.299, 2 * 0.564 * -0.587, 2 * 0.564 * (1 - 0.114)],
            [2 * 0.713 * (1 - 0.299), 2 * 0.713 * -0.587, 2 * 0.713 * -0.114],
        ],
        np.float64,
    )
    w1 = np.zeros((96, 96))  # [(c', y), (c, v)]
    for cp in range(3):
        for c in range(3):
            w1[cp * 32 : (cp + 1) * 32, c * 32 : (c + 1) * 32] = a2[c, cp] * cs.T
    w2 = np.zeros((128, 128))  # [(gxl, x'), (gxl, u)] block diagonal over gxl
    for g in range(4):
        w2[g * 32 : (g + 1) * 32, g * 32 : (g + 1) * 32] = cs.T
    return w1.astype(np.float16), w2.astype(np.float16)


def _build_program():
    nc = bacc.Bacc(trn_type="TRN2")
    # host pre-groups 4 block-rows so each partition's 4KB is contiguous
    x = nc.dram_tensor("x", [GROUPS, 96, 4, 512], F16, kind="ExternalInput")
    w1 = nc.dram_tensor("w1", [96, 96], F16, kind="ExternalInput")
    w2 = nc.dram_tensor("w2", [128, 128], F16, kind="ExternalInput")
    out = nc.dram_tensor("out", [GROUPS, 128, 1536], F16, kind="ExternalOutput")

    with tile.TileContext(nc) as tc:
        with (
            tc.tile_pool(name="const", bufs=1) as constp,
            tc.tile_pool(name="pin", bufs=3) as pin,
            tc.tile_pool(name="pmid", bufs=8) as pmid,
            tc.tile_pool(name="pout", bufs=3) as pout,
            tc.tile_pool(name="psA", bufs=4, space="PSUM") as psA,
            tc.tile_pool(name="psB", bufs=2, space="PSUM") as psB,
        ):
            w1s = constp.tile([96, 96], F16)
            w2s = constp.tile([128, 128], F16)
            nc.sync.dma_start(w1s[:], w1[:])
            nc.sync.dma_start(w2s[:], w2[:])

            for g in range(GROUPS):
                img4 = pin.tile([96, 2048], F16, tag="img4")
                nc.sync.dma_start(
                    img4[:].rearrange("p (r x) -> p r x", r=4), x[g]
                )
                osb4 = pout.tile([128, 1536], F16, tag="osb4")
                o2p = None
                for j in range(4):
                    it = 4 * g + j
                    # stage 1 (stationary swap): 4 chunk slots, one PSUM bank
                    t1T = psA.tile([128, 512], F32, tag="t1T")
                    for k in range(4):
                        nc.tensor.matmul(
                            t1T[:, k * 128 : k * 128 + 96],
                            img4[:, j * 512 + k * 128 : j * 512 + (k + 1) * 128],
                            w1s[:],
                            start=True,
                            stop=True,
                        )
                    # cast1: pack 4 chunk slots -> contiguous fp16 [128, 384]
                    t1s = pmid.tile([128, 384], F16, tag="t1s")
                    src = t1T[:].rearrange("p (k s) -> p k s", k=4)[:, :, 0:96]
                    dst = t1s[:].rearrange("p (k s) -> p k s", k=4)
                    if it % 2 == 0:
                        nc.scalar.copy(dst, src)
                    else:
                        nc.vector.tensor_copy(dst, src)
                    # stage 2: one matmul; two iterations share a 2-bank tile
                    if j % 2 == 0:
                        o2p = psB.tile([128, 1024], F32, tag="o2p")
                    nc.tensor.matmul(
                        o2p[:, (j % 2) * 512 : (j % 2) * 512 + 384],
                        w2s[:],
                        t1s[:],
                        start=True,
                        stop=True,
                    )
                    # cast2 covers both halves once the pair is done
                    if j % 2 == 1:
                        csrc = o2p[:].rearrange("p (r s) -> p r s", r=2)[:, :, 0:384]
                        cdst = osb4[:, (j - 1) * 384 : (j + 1) * 384].rearrange(
                            "p (r s) -> p r s", r=2
                        )
                        if j == 1:
                            nc.vector.tensor_copy(cdst, csrc)
                        else:
                            nc.scalar.copy(cdst, csrc)
                # one output DMA per 4 iterations, on the GpSimd DGE queue
                nc.gpsimd.dma_start(out[g], osb4[:])

    nc.finalize()
    return nc


def _get_program():
    if "nc" not in _STATE:
        _STATE["nc"] = _build_program()
        _STATE["consts"] = _constants()
        _STATE["sort_idx"] = _sort_idx()
    return _STATE["nc"]


def kernel(**inputs):
    global LAST_RESULT
    rgb = np.asarray(inputs["rgb_images_batch"], np.float32)
    assert rgb.shape == (N_CORES * B_PER_CORE, 3, 512, 512)
    B = N_CORES * B_PER_CORE
    xs = rgb.reshape(B, 3, NH, 32, 512).transpose(0, 2, 1, 3, 4)
    xs = (np.ascontiguousarray(xs).reshape(B, NH, 96, 512)
          - np.float32(0.5)).astype(np.float16)
    # group 4 block-rows with the partition dim outermost: [B, 4g, 96, 4r, 512]
    xs = np.ascontiguousarray(xs.reshape(B, NH // 4, 4, 96, 512).transpose(0, 1, 3, 2, 4))
    xs = xs.reshape(B, NH // 4, 96, 4, 512)
    nc = _get_program()
    w1, w2 = _STATE["consts"]
    sort_idx = _STATE["sort_idx"]

    in_maps = [
        {
            "x": xs[c * B_PER_CORE : (c + 1) * B_PER_CORE].reshape(GROUPS, 96, 4, 512),
            "w1": w1,
            "w2": w2,
        }
        for c in range(N_CORES)
    ]
    trace = os.environ.get("KERNEL_TRACE", "0") == "1"
    res = run_bass_kernel_spmd(
        nc, in_maps, core_ids=list(range(N_CORES)), trace=trace
    )
    LAST_RESULT = res

    outs = []
    for c in range(N_CORES):
        dev = res.results[c]["out"].astype(np.float32)  # [16, 128, 1536]
        dev = dev.reshape(GROUPS, 128, 4, 384).transpose(0, 2, 1, 3)
        dev = dev.reshape(ITERS, 128, 384)
        # [it=(b,br), p=(gxl,u), col=(kk, c, v)]
        a = dev.reshape(B_PER_CORE, NH, 4, 32, 4, 3, 32)  # b,br,gxl,u,kk,c,v
        a = a.transpose(0, 5, 6, 3, 1, 4, 2)  # b,c,v,u,br,kk,gxl
        a = np.ascontiguousarray(a).reshape(B_PER_CORE, 3, 1024, NH, NH)
        a = a[:, :, sort_idx, :, :]
        outs.append(a.reshape(B_PER_CORE, 3 * 1024, NH, NH))
    return np.concatenate(outs, axis=0)
